# revision 7
# baseline (speedup 1.0000x reference)
"""HDC sigmoid-attention kernel for Trainium2 (8 NeuronCores).

Problem: out = causal_sigmoid_attn(q, k, v) where q/k/v = x * sign_vec(bv_*),
x: [4, 4096, 1024] f32.  Returns (out, k, v) like the reference.

Sharding: 8 cores = 4 batches x 2 row-parity groups.  Core (b, h) handles
batch b, rows {t : t % 2 == h}.  Row-parity interleaving makes the causal
work profile identical on every core, so one SPMD program serves all 8.

Per core: 2048 rows as 8 t-blocks (J=0..7) of 256 local rows; t-block J
covers global rows {512J + 2m + h}.  Causal extent of block J is s-chunks
0..4J+3 (chunk = 128 s values); the top 4 chunks are diagonal and get a
0/1 mask (host-precomputed, J-independent thanks to the parity trick).

Matmul 1 (scores^T) in FP8 e4m3 with DoubleRow (2 d-subtiles per pass,
~2x the bf16 row rate):
  psum[s=128, t=256] += kT[d=128, 2, s=128].T @ qT[d=128, 2, t=256]
q/k are scaled by 16*alpha on host before the e4m3 cast; the 1/(16*16)
plus the reference's 4/sqrt(D) fold into the activation scale.

Sigmoid split: sigma(z) = 0.5 + 0.5*tanh(z/2).  The 0.5*cumsum(v) term is
exact host-side work (prefix sum over s); the device only computes
H = sum_s tanh(z/2) * v.  tanh in [-1,1] is ~4x smaller in RMS than
sigma in [0,1], so fp8-quantizing BOTH mm2 operands costs only ~1% rel
err (vs ~2.5% without the split).  ACT emits tanh directly as e4m3.

Matmul 2 (H) in FP8 e4m3 DoubleRow over chunk PAIRS (2 s-chunks = 256
contraction per pass):
  psum[t=128, d=512] += gT[s=128, 2, t=128].T @ v8[s=128, 2, d=512]
The attn tile's natural [s, pair-half, t] layout IS the DoubleRow
interleave; v8 is host-packed as [s, pair, 2, d].  Host output:
out = 0.5 * (cumsum(v) + H).

kT (fp8, 4MB) and v8 (fp8, 4MB) are fully SBUF-resident; no streaming.
H is stored fp16 and combined on host in f32 (rel-err impact ~1e-4).
"""

import numpy as np
import ml_dtypes

import concourse.bass as bass
import concourse.bacc as bacc
import concourse.mybir as mybir
import concourse.tile as tile
from concourse.bass_utils import run_bass_kernel_spmd

B, T, D = 4, 4096, 1024
P = 128
NJ = 8          # t-blocks per core
TB = 256        # local rows per t-block
NC = 32         # s-chunks per batch
NPAIR = NC // 2

F32 = mybir.dt.float32
BF16 = mybir.dt.bfloat16
FP16 = mybir.dt.float16
FP8 = mybir.dt.float8e4
NP_FP16 = np.float16
NP_FP8 = ml_dtypes.float8_e4m3

QK_SCALE = 16.0                       # host pre-scale before e4m3 cast
ACT_SCALE = 0.125 / (QK_SCALE * QK_SCALE)

_nc_cache = {}
TRACE = False  # set True (e.g. from test.py) to collect an NTFF profile


def _build_nc(reps=1):
    nc = bacc.Bacc("TRN2", debug=False, target_bir_lowering=False, num_devices=8)

    qT_d = nc.dram_tensor("qT", [NJ, P, 8, TB], FP8, kind="ExternalInput")
    kT_d = nc.dram_tensor("kT", [NC, P, 8, P], FP8, kind="ExternalInput")
    v_d = nc.dram_tensor("v", [P, NPAIR, 2, 1024], FP8, kind="ExternalInput")
    mk_d = nc.dram_tensor("masks", [2, P, 2, TB], FP16, kind="ExternalInput")
    out_d = nc.dram_tensor("out_loc", [2048, D], FP16, kind="ExternalOutput")

    with tile.TileContext(nc) as tc:
        with (
            tc.tile_pool(name="vres", bufs=1) as vpool,
            tc.tile_pool(name="kres", bufs=1) as krespool,
            tc.tile_pool(name="qt", bufs=2) as qpool,
            tc.tile_pool(name="attn", bufs=12) as apool,
            tc.tile_pool(name="mask", bufs=1) as mpool,
            tc.tile_pool(name="ostage", bufs=4) as opool,
            tc.tile_pool(name="ps_s", bufs=4, space=bass.MemorySpace.PSUM) as pspool,
            tc.tile_pool(name="ps_o", bufs=1, space=bass.MemorySpace.PSUM) as popool,
        ):
            v_sb = {}

            def get_v(pp):
                # lazy one-time load so early t-blocks' inputs win the DMA queue
                if pp not in v_sb:
                    vt = vpool.tile([P, 2, 1024], FP8, tag=f"v{pp}", name=f"v{pp}")
                    nc.sync.dma_start(out=vt[:], in_=v_d[:, pp])
                    v_sb[pp] = vt
                return v_sb[pp]

            k_sb = {}

            def get_k(c):
                if c not in k_sb:
                    kt = krespool.tile([P, 8, P], FP8, tag=f"k{c}", name=f"k{c}")
                    # chunks 0..7 (J0/J1's working set) ride two queues so
                    # the DMA-latency-bound start drains 2x faster
                    q = nc.scalar if (c < 8 and c % 2 == 1) else nc.gpsimd
                    q.dma_start(out=kt[:], in_=kT_d[c])
                    k_sb[c] = kt
                return k_sb[c]

            masks = []
            for mi in range(2):
                mt = mpool.tile([P, 2, TB], FP16, tag=f"mask{mi}")
                nc.scalar.dma_start(out=mt[:], in_=mk_d[mi])
                masks.append(mt)

            # PE warm-up: the HAM clock gate needs ~3.4us of sustained PE
            # activity before the array runs at 2.4 GHz.  The first input
            # DMAs take ~1.2us to land; fill that window with throwaway
            # matmuls on a zeroed scratch tile so the ramp starts early.
            wsb = mpool.tile([P, 2, TB], FP8, tag="warm")
            nc.gpsimd.memset(wsb[:], 0.0)
            wps = pspool.tile([P, 2, TB], F32, tag="scores")
            for i in range(8):
                nc.tensor.matmul(
                    wps[:, 0, :],
                    wsb[:, :, :P],
                    wsb[:],
                    start=(i == 0),
                    stop=(i == 7),
                    perf_mode=mybir.MatmulPerfMode.DoubleRow,
                )

            import contextlib
            if reps > 1:
                for c in range(NC):
                    get_k(c)
                for pp in range(NPAIR):
                    get_v(pp)
            rep_ctx = tc.For_i(0, reps, 1) if reps > 1 else contextlib.nullcontext()
            with rep_ctx:
                _kernel_body(nc, tc, qT_d, get_k, get_v, out_d, masks,
                             qpool, apool, opool, pspool, popool)

    nc.compile()
    return nc


def _kernel_body(nc, tc, qT_d, get_k, get_v, out_d, masks,
                 qpool, apool, opool, pspool, popool):
    def emit_out(J, tt, accs, opool, out_d):
        ot = opool.tile([P, 1024], FP16, tag="ostage")
        for dd in range(2):
            if J == NJ - 1 and tt == 1 and dd == 1:
                # tail: drain the very last accumulator on the (now idle)
                # scalar engine so both copies run in parallel
                nc.scalar.activation(
                    ot[:, dd * 512:(dd + 1) * 512], accs[tt * 2 + dd][:],
                    mybir.ActivationFunctionType.Copy,
                )
            else:
                nc.vector.tensor_copy(
                    ot[:, dd * 512:(dd + 1) * 512], accs[tt * 2 + dd][:]
                )
        # the last block's stores ride the sync queue: its input loads are all
        # issued by then, and sync's end-of-kernel drain is ~3us cheaper than
        # gpsimd's
        q = nc.sync if J == NJ - 1 else nc.gpsimd
        q.dma_start(
            out=out_d[J * TB + tt * 128: J * TB + (tt + 1) * 128, :],
            in_=ot[:],
        )

    for J in range(NJ):
        qt = qpool.tile([P, 8, TB], FP8, tag="qt")
        nc.sync.dma_start(out=qt[:], in_=qT_d[J])
        ns = 4 * J + 4
        # prefetch: k chunks for the NEXT t-block on the gpsimd queue (ahead
        # of this block's out-DMA triggers), this block's v pairs on sync.
        for c in range(ns):
            get_k(c)
        for pp in range(2 * J + 2):
            get_v(pp)
        if J + 1 < NJ:
            for c in range(4 * J + 4, 4 * J + 8):
                get_k(c)
        # chunk pair 2J+1 (chunks 4J+2, 4J+3) is fully causal-masked for the
        # tt=0 row half (local rows 0..127 <-> global 2m+h < 256): skip its
        # tt=0 matmuls and drain the tt=0 accumulators one pair early.
        accs = []
        for i in range(4):
            acc_t = popool.tile([P, 512], F32, tag=f"acc{i}", name=f"acc{i}_{J}")
            accs.append(acc_t)
        # chunk PAIRS share one [128, 2, TB] scores psum tile: a PSUM zero
        # region is the whole 2KB bank, so one accumulation group (start on
        # the first matmul of the pair) covers both halves.  The [s, half, t]
        # layout doubles as the DoubleRow interleave for mm2.
        for pp in range(ns // 2):
            ps = pspool.tile([P, 2, TB], F32, tag="scores")
            for half in range(2):
                c = 2 * pp + half
                kt = get_k(c)
                for ccp in range(4):
                    nc.tensor.matmul(
                        ps[:, half, :],
                        kt[:, 2 * ccp:2 * ccp + 2, :],
                        qt[:, 2 * ccp:2 * ccp + 2, :],
                        start=(half == 0 and ccp == 0),
                        stop=(half == 1 and ccp == 3),
                        perf_mode=mybir.MatmulPerfMode.DoubleRow,
                    )
            at = apool.tile([P, 2, TB], FP8, tag="attn")
            nc.scalar.activation(
                at[:], ps[:],
                mybir.ActivationFunctionType.Tanh,
                scale=ACT_SCALE / 2,
            )
            if pp == 2 * J:
                nc.vector.tensor_mul(at[:], at[:], masks[0][:])
            elif pp == 2 * J + 1:
                nc.vector.tensor_mul(at[:], at[:], masks[1][:])
            vt = get_v(pp)
            tts = (0, 1) if pp <= 2 * J else (1,)
            for tt in tts:
                for dd in range(2):
                    nc.tensor.matmul(
                        accs[tt * 2 + dd][:],
                        at[:, :, tt * 128:(tt + 1) * 128],
                        vt[:, :, dd * 512:(dd + 1) * 512],
                        start=(pp == 0),
                        stop=(pp == (2 * J if tt == 0 else 2 * J + 1)),
                        perf_mode=mybir.MatmulPerfMode.DoubleRow,
                    )
            if pp == 2 * J:
                emit_out(J, 0, accs, opool, out_d)
        emit_out(J, 1, accs, opool, out_d)


def _get_nc(reps=1):
    key = ("nc", reps)
    if key not in _nc_cache:
        _nc_cache[key] = _build_nc(reps)
    return _nc_cache[key]


def _sign_vec(w):
    w = np.asarray(w, np.float32)
    alpha = np.float32(np.mean(np.abs(w), dtype=np.float32))
    hard = (alpha * np.sign(w)).astype(np.float32)
    hard = np.where(hard == 0, alpha, hard).astype(np.float32)
    return hard


def _rows_of(h):
    l = np.arange(2048)
    return 512 * (l // 256) + 2 * (l % 256) + h


def _masks_of(h):
    m = np.arange(TB)[None, :]      # local row in t-block
    p = np.arange(P)[:, None]       # s within chunk
    out = np.empty((4, P, TB), np.float32)
    for mi in range(4):
        out[mi] = ((2 * m + h) >= (128 * mi + p)).astype(np.float32)
    # paired layout matching the chunk-pair attn tiles: [2, P, 2, TB] with
    # pair 0 = diag chunks (mi0|mi1), pair 1 = (mi2|mi3)
    return np.stack(
        [np.stack([out[0], out[1]], axis=1),
         np.stack([out[2], out[3]], axis=1)], axis=0)


def kernel(x, bv_q, bv_k, bv_v):
    x = np.ascontiguousarray(np.asarray(x, np.float32))
    sq = _sign_vec(bv_q)
    sk = _sign_vec(bv_k)
    sv = _sign_vec(bv_v)

    k_full = (x * sk).astype(np.float32)
    v_full = (x * sv).astype(np.float32)

    q_s = (x * (sq * QK_SCALE)).astype(np.float32)
    k_s = (x * (sk * QK_SCALE)).astype(np.float32)

    # exact host-side half of the sigmoid split: 0.5 * cumsum(v)
    prefix_v = np.cumsum(v_full, axis=1, dtype=np.float64).astype(np.float32)

    nc = _get_nc()
    rows = {h: _rows_of(h) for h in range(2)}
    mks = {h: _masks_of(h) for h in range(2)}

    in_maps = []
    for core in range(8):
        b, h = core // 2, core % 2
        qrows = q_s[b][rows[h]]                         # [2048, 1024]
        qT_host = np.ascontiguousarray(
            qrows.reshape(NJ, TB, 8, P).transpose(0, 3, 2, 1)
        )                                               # [NJ, P, 8, TB]
        kT_host = np.ascontiguousarray(
            k_s[b].reshape(NC, P, 8, P).transpose(0, 3, 2, 1)
        )                                               # [NC, P, 8, P]
        v_host = np.ascontiguousarray(
            v_full[b].reshape(NPAIR, 2, P, 1024).transpose(2, 0, 1, 3)
        )                                               # [P, NPAIR, 2, 1024]
        in_maps.append({
            "qT": qT_host.astype(NP_FP8),
            "kT": kT_host.astype(NP_FP8),
            "v": v_host.astype(NP_FP8),
            "masks": mks[h].astype(NP_FP16),
        })

    bkr = run_bass_kernel_spmd(nc, in_maps, list(range(8)), trace=TRACE)
    _nc_cache["last"] = bkr
    res = bkr.results

    out = np.empty((B, T, D), np.float32)
    for core in range(8):
        b, h = core // 2, core % 2
        H = np.asarray(res[core]["out_loc"]).astype(np.float32)
        out[b, rows[h]] = 0.5 * (prefix_v[b][rows[h]] + H)

    return out, k_full, v_full


# revision 10
# speedup vs baseline: 1.0021x; 1.0021x over previous
"""HDC sigmoid-attention kernel for Trainium2 (8 NeuronCores).

Problem: out = causal_sigmoid_attn(q, k, v) where q/k/v = x * sign_vec(bv_*),
x: [4, 4096, 1024] f32.  Returns (out, k, v) like the reference.

Sharding: 8 cores = 4 batches x 2 row-parity groups.  Core (b, h) handles
batch b, rows {t : t % 2 == h}.  Row-parity interleaving makes the causal
work profile identical on every core, so one SPMD program serves all 8.

Per core: 2048 rows as 8 t-blocks (J=0..7) of 256 local rows; t-block J
covers global rows {512J + 2m + h}.  Causal extent of block J is s-chunks
0..4J+3 (chunk = 128 s values); the top 4 chunks are diagonal and get a
0/1 mask (host-precomputed, J-independent thanks to the parity trick).

Matmul 1 (scores^T) in FP8 e4m3 with DoubleRow (2 d-subtiles per pass,
~2x the bf16 row rate):
  psum[s=128, t=256] += kT[d=128, 2, s=128].T @ qT[d=128, 2, t=256]
q/k are scaled by 16*alpha on host before the e4m3 cast; the 1/(16*16)
plus the reference's 4/sqrt(D) fold into the activation scale.

Sigmoid split: sigma(z) = 0.5 + 0.5*tanh(z/2).  The 0.5*cumsum(v) term is
exact host-side work (prefix sum over s); the device only computes
H = sum_s tanh(z/2) * v.  tanh in [-1,1] is ~4x smaller in RMS than
sigma in [0,1], so fp8-quantizing BOTH mm2 operands costs only ~1% rel
err (vs ~2.5% without the split).  ACT emits tanh directly as e4m3.

Matmul 2 (H) in FP8 e4m3 DoubleRow over chunk PAIRS (2 s-chunks = 256
contraction per pass):
  psum[t=128, d=512] += gT[s=128, 2, t=128].T @ v8[s=128, 2, d=512]
The attn tile's natural [s, pair-half, t] layout IS the DoubleRow
interleave; v8 is host-packed as [s, pair, 2, d].  Host output:
out = 0.5 * (cumsum(v) + H).

kT (fp8, 4MB) and v8 (fp8, 4MB) are fully SBUF-resident; no streaming.
H is stored fp16 and combined on host in f32 (rel-err impact ~1e-4).
"""

import numpy as np
import ml_dtypes

import concourse.bass as bass
import concourse.bacc as bacc
import concourse.mybir as mybir
import concourse.tile as tile
from concourse.bass_utils import run_bass_kernel_spmd

B, T, D = 4, 4096, 1024
P = 128
NJ = 8          # t-blocks per core
TB = 256        # local rows per t-block
NC = 32         # s-chunks per batch
NPAIR = NC // 2

F32 = mybir.dt.float32
BF16 = mybir.dt.bfloat16
FP16 = mybir.dt.float16
FP8 = mybir.dt.float8e4
NP_FP16 = np.float16
NP_FP8 = ml_dtypes.float8_e4m3

QK_SCALE = 16.0                       # host pre-scale before e4m3 cast
ACT_SCALE = 0.125 / (QK_SCALE * QK_SCALE)

_nc_cache = {}
TRACE = False  # set True (e.g. from test.py) to collect an NTFF profile


def _build_nc(reps=1):
    nc = bacc.Bacc("TRN2", debug=False, target_bir_lowering=False, num_devices=8)

    qT_d = nc.dram_tensor("qT", [NJ, P, 8, TB], FP8, kind="ExternalInput")
    kT_d = nc.dram_tensor("kT", [NC, P, 8, P], FP8, kind="ExternalInput")
    v_d = nc.dram_tensor("v", [P, NPAIR, 2, 1024], FP8, kind="ExternalInput")
    mk_d = nc.dram_tensor("masks", [2, P, 2, TB], FP16, kind="ExternalInput")
    out_d = nc.dram_tensor("out_loc", [2048, D], FP16, kind="ExternalOutput")

    with tile.TileContext(nc) as tc:
        with (
            tc.tile_pool(name="vres", bufs=1) as vpool,
            tc.tile_pool(name="kres", bufs=1) as krespool,
            tc.tile_pool(name="qt", bufs=2) as qpool,
            tc.tile_pool(name="attn", bufs=12) as apool,
            tc.tile_pool(name="mask", bufs=1) as mpool,
            tc.tile_pool(name="ostage", bufs=4) as opool,
            tc.tile_pool(name="ps_s", bufs=4, space=bass.MemorySpace.PSUM) as pspool,
            tc.tile_pool(name="ps_o", bufs=1, space=bass.MemorySpace.PSUM) as popool,
        ):
            v_sb = {}

            def get_v(pp):
                # lazy one-time load so early t-blocks' inputs win the DMA queue
                if pp not in v_sb:
                    vt = vpool.tile([P, 2, 1024], FP8, tag=f"v{pp}", name=f"v{pp}")
                    if pp == 0:
                        # split so mm2(J0,pp0,dd0) can start off the first half
                        for dd in range(2):
                            nc.sync.dma_start(
                                out=vt[:, :, dd * 512:(dd + 1) * 512],
                                in_=v_d[:, pp, :, dd * 512:(dd + 1) * 512],
                            )
                    else:
                        nc.sync.dma_start(out=vt[:], in_=v_d[:, pp])
                    v_sb[pp] = vt
                return v_sb[pp]

            k_sb = {}

            def get_k(c):
                if c not in k_sb:
                    kt = krespool.tile([P, 8, P], FP8, tag=f"k{c}", name=f"k{c}")
                    if c == 0:
                        # split the first k chunk so mm1(J0,c0,ccp0) can
                        # start as soon as its 32KB slice lands
                        for p in range(4):
                            nc.gpsimd.dma_start(
                                out=kt[:, 2 * p:2 * p + 2, :],
                                in_=kT_d[c][:, 2 * p:2 * p + 2, :],
                            )
                    else:
                        nc.gpsimd.dma_start(out=kt[:], in_=kT_d[c])
                    k_sb[c] = kt
                return k_sb[c]

            masks = []
            for mi in range(2):
                mt = mpool.tile([P, 2, TB], FP16, tag=f"mask{mi}")
                nc.scalar.dma_start(out=mt[:], in_=mk_d[mi])
                masks.append(mt)

            # PE warm-up: the HAM clock gate needs ~3.4us of sustained PE
            # activity before the array runs at 2.4 GHz.  The first input
            # DMAs take ~1.2us to land; fill that window with throwaway
            # matmuls on a zeroed scratch tile so the ramp starts early.
            wsb = mpool.tile([P, 2, TB], FP8, tag="warm")
            nc.gpsimd.memset(wsb[:], 0.0)
            wps = pspool.tile([P, 2, TB], F32, tag="scores")
            for i in range(8):
                nc.tensor.matmul(
                    wps[:, 0, :],
                    wsb[:, :, :P],
                    wsb[:],
                    start=(i == 0),
                    stop=(i == 7),
                    perf_mode=mybir.MatmulPerfMode.DoubleRow,
                )

            import contextlib
            if reps > 1:
                for c in range(NC):
                    get_k(c)
                for pp in range(NPAIR):
                    get_v(pp)
            rep_ctx = tc.For_i(0, reps, 1) if reps > 1 else contextlib.nullcontext()
            with rep_ctx:
                _kernel_body(nc, tc, qT_d, get_k, get_v, out_d, masks,
                             qpool, apool, opool, pspool, popool)

    nc.compile()
    return nc


def _kernel_body(nc, tc, qT_d, get_k, get_v, out_d, masks,
                 qpool, apool, opool, pspool, popool):
    def emit_out(J, tt, accs, opool, out_d):
        ot = opool.tile([P, 1024], FP16, tag="ostage")
        for dd in range(2):
            if J == NJ - 1 and tt == 1 and dd == 1:
                # tail: drain the very last accumulator on the (now idle)
                # scalar engine so both copies run in parallel
                nc.scalar.activation(
                    ot[:, dd * 512:(dd + 1) * 512], accs[tt * 2 + dd][:],
                    mybir.ActivationFunctionType.Copy,
                )
            else:
                nc.vector.tensor_copy(
                    ot[:, dd * 512:(dd + 1) * 512], accs[tt * 2 + dd][:]
                )
        # the last block's stores ride the sync queue: its input loads are all
        # issued by then, and sync's end-of-kernel drain is ~3us cheaper than
        # gpsimd's
        q = nc.sync if J == NJ - 1 else nc.gpsimd
        q.dma_start(
            out=out_d[J * TB + tt * 128: J * TB + (tt + 1) * 128, :],
            in_=ot[:],
        )

    for J in range(NJ):
        qt = qpool.tile([P, 8, TB], FP8, tag="qt")
        if J == 0:
            for p in range(4):
                nc.sync.dma_start(
                    out=qt[:, 2 * p:2 * p + 2, :],
                    in_=qT_d[J][:, 2 * p:2 * p + 2, :],
                )
        else:
            nc.sync.dma_start(out=qt[:], in_=qT_d[J])
        ns = 4 * J + 4
        # prefetch: k chunks for the NEXT t-block on the gpsimd queue (ahead
        # of this block's out-DMA triggers), this block's v pairs on sync.
        for c in range(ns):
            get_k(c)
        for pp in range(2 * J + 2):
            get_v(pp)
        if J + 1 < NJ:
            for c in range(4 * J + 4, 4 * J + 8):
                get_k(c)
        # chunk pair 2J+1 (chunks 4J+2, 4J+3) is fully causal-masked for the
        # tt=0 row half (local rows 0..127 <-> global 2m+h < 256): skip its
        # tt=0 matmuls and drain the tt=0 accumulators one pair early.
        accs = []
        for i in range(4):
            acc_t = popool.tile([P, 512], F32, tag=f"acc{i}", name=f"acc{i}_{J}")
            accs.append(acc_t)
        # chunk PAIRS share one [128, 2, TB] scores psum tile: a PSUM zero
        # region is the whole 2KB bank, so one accumulation group (start on
        # the first matmul of the pair) covers both halves.  The [s, half, t]
        # layout doubles as the DoubleRow interleave for mm2.
        for pp in range(ns // 2):
            ps = pspool.tile([P, 2, TB], F32, tag="scores")
            for half in range(2):
                c = 2 * pp + half
                kt = get_k(c)
                for ccp in range(4):
                    nc.tensor.matmul(
                        ps[:, half, :],
                        kt[:, 2 * ccp:2 * ccp + 2, :],
                        qt[:, 2 * ccp:2 * ccp + 2, :],
                        start=(half == 0 and ccp == 0),
                        stop=(half == 1 and ccp == 3),
                        perf_mode=mybir.MatmulPerfMode.DoubleRow,
                    )
            at = apool.tile([P, 2, TB], FP8, tag="attn")
            nc.scalar.activation(
                at[:], ps[:],
                mybir.ActivationFunctionType.Tanh,
                scale=ACT_SCALE / 2,
            )
            if pp == 2 * J:
                nc.vector.tensor_mul(at[:], at[:], masks[0][:])
            elif pp == 2 * J + 1:
                nc.vector.tensor_mul(at[:], at[:], masks[1][:])
            vt = get_v(pp)
            tts = (0, 1) if pp <= 2 * J else (1,)
            for tt in tts:
                for dd in range(2):
                    nc.tensor.matmul(
                        accs[tt * 2 + dd][:],
                        at[:, :, tt * 128:(tt + 1) * 128],
                        vt[:, :, dd * 512:(dd + 1) * 512],
                        start=(pp == 0),
                        stop=(pp == (2 * J if tt == 0 else 2 * J + 1)),
                        perf_mode=mybir.MatmulPerfMode.DoubleRow,
                    )
            if pp == 2 * J:
                emit_out(J, 0, accs, opool, out_d)
        emit_out(J, 1, accs, opool, out_d)


def _get_nc(reps=1):
    key = ("nc", reps)
    if key not in _nc_cache:
        _nc_cache[key] = _build_nc(reps)
    return _nc_cache[key]


def _sign_vec(w):
    w = np.asarray(w, np.float32)
    alpha = np.float32(np.mean(np.abs(w), dtype=np.float32))
    hard = (alpha * np.sign(w)).astype(np.float32)
    hard = np.where(hard == 0, alpha, hard).astype(np.float32)
    return hard


def _rows_of(h):
    l = np.arange(2048)
    return 512 * (l // 256) + 2 * (l % 256) + h


def _masks_of(h):
    m = np.arange(TB)[None, :]      # local row in t-block
    p = np.arange(P)[:, None]       # s within chunk
    out = np.empty((4, P, TB), np.float32)
    for mi in range(4):
        out[mi] = ((2 * m + h) >= (128 * mi + p)).astype(np.float32)
    # paired layout matching the chunk-pair attn tiles: [2, P, 2, TB] with
    # pair 0 = diag chunks (mi0|mi1), pair 1 = (mi2|mi3)
    return np.stack(
        [np.stack([out[0], out[1]], axis=1),
         np.stack([out[2], out[3]], axis=1)], axis=0)


def kernel(x, bv_q, bv_k, bv_v):
    x = np.ascontiguousarray(np.asarray(x, np.float32))
    sq = _sign_vec(bv_q)
    sk = _sign_vec(bv_k)
    sv = _sign_vec(bv_v)

    k_full = (x * sk).astype(np.float32)
    v_full = (x * sv).astype(np.float32)

    q_s = (x * (sq * QK_SCALE)).astype(np.float32)
    k_s = (x * (sk * QK_SCALE)).astype(np.float32)

    # exact host-side half of the sigmoid split: 0.5 * cumsum(v)
    prefix_v = np.cumsum(v_full, axis=1, dtype=np.float64).astype(np.float32)

    nc = _get_nc()
    rows = {h: _rows_of(h) for h in range(2)}
    mks = {h: _masks_of(h) for h in range(2)}

    in_maps = []
    for core in range(8):
        b, h = core // 2, core % 2
        qrows = q_s[b][rows[h]]                         # [2048, 1024]
        qT_host = np.ascontiguousarray(
            qrows.reshape(NJ, TB, 8, P).transpose(0, 3, 2, 1)
        )                                               # [NJ, P, 8, TB]
        kT_host = np.ascontiguousarray(
            k_s[b].reshape(NC, P, 8, P).transpose(0, 3, 2, 1)
        )                                               # [NC, P, 8, P]
        v_host = np.ascontiguousarray(
            v_full[b].reshape(NPAIR, 2, P, 1024).transpose(2, 0, 1, 3)
        )                                               # [P, NPAIR, 2, 1024]
        in_maps.append({
            "qT": qT_host.astype(NP_FP8),
            "kT": kT_host.astype(NP_FP8),
            "v": v_host.astype(NP_FP8),
            "masks": mks[h].astype(NP_FP16),
        })

    bkr = run_bass_kernel_spmd(nc, in_maps, list(range(8)), trace=TRACE)
    _nc_cache["last"] = bkr
    res = bkr.results

    out = np.empty((B, T, D), np.float32)
    for core in range(8):
        b, h = core // 2, core % 2
        H = np.asarray(res[core]["out_loc"]).astype(np.float32)
        out[b, rows[h]] = 0.5 * (prefix_v[b][rows[h]] + H)

    return out, k_full, v_full


# revision 14
# speedup vs baseline: 1.0022x; 1.0002x over previous
"""HDC sigmoid-attention kernel for Trainium2 (8 NeuronCores).

Problem: out = causal_sigmoid_attn(q, k, v) where q/k/v = x * sign_vec(bv_*),
x: [4, 4096, 1024] f32.  Returns (out, k, v) like the reference.

Sharding: 8 cores = 4 batches x 2 row-parity groups.  Core (b, h) handles
batch b, rows {t : t % 2 == h}.  Row-parity interleaving makes the causal
work profile identical on every core, so one SPMD program serves all 8.

Per core: 2048 rows as 8 t-blocks (J=0..7) of 256 local rows; t-block J
covers global rows {512J + 2m + h}.  Causal extent of block J is s-chunks
0..4J+3 (chunk = 128 s values); the top 4 chunks are diagonal and get a
0/1 mask (host-precomputed, J-independent thanks to the parity trick).

Matmul 1 (scores^T) in FP8 e4m3 with DoubleRow (2 d-subtiles per pass,
~2x the bf16 row rate):
  psum[s=128, t=256] += kT[d=128, 2, s=128].T @ qT[d=128, 2, t=256]
q/k are scaled by 16*alpha on host before the e4m3 cast; the 1/(16*16)
plus the reference's 4/sqrt(D) fold into the activation scale.

Sigmoid split: sigma(z) = 0.5 + 0.5*tanh(z/2).  The 0.5*cumsum(v) term is
exact host-side work (prefix sum over s); the device only computes
H = sum_s tanh(z/2) * v.  tanh in [-1,1] is ~4x smaller in RMS than
sigma in [0,1], so fp8-quantizing BOTH mm2 operands costs only ~1% rel
err (vs ~2.5% without the split).  ACT emits tanh directly as e4m3.

Matmul 2 (H) in FP8 e4m3 DoubleRow over chunk PAIRS (2 s-chunks = 256
contraction per pass):
  psum[t=128, d=512] += gT[s=128, 2, t=128].T @ v8[s=128, 2, d=512]
The attn tile's natural [s, pair-half, t] layout IS the DoubleRow
interleave; v8 is host-packed as [s, pair, 2, d].  Host output:
out = 0.5 * (cumsum(v) + H).

kT (fp8, 4MB) and v8 (fp8, 4MB) are fully SBUF-resident; no streaming.
H is stored fp16 and combined on host in f32 (rel-err impact ~1e-4).
"""

import numpy as np
import ml_dtypes

import concourse.bass as bass
import concourse.bacc as bacc
import concourse.mybir as mybir
import concourse.tile as tile
from concourse.bass_utils import run_bass_kernel_spmd

B, T, D = 4, 4096, 1024
P = 128
NJ = 8          # t-blocks per core
TB = 256        # local rows per t-block
NC = 32         # s-chunks per batch
NPAIR = NC // 2

F32 = mybir.dt.float32
BF16 = mybir.dt.bfloat16
FP16 = mybir.dt.float16
FP8 = mybir.dt.float8e4
NP_FP16 = np.float16
NP_FP8 = ml_dtypes.float8_e4m3

QK_SCALE = 16.0                       # host pre-scale before e4m3 cast
ACT_SCALE = 0.125 / (QK_SCALE * QK_SCALE)

_nc_cache = {}
TRACE = False  # set True (e.g. from test.py) to collect an NTFF profile


def _build_nc(reps=1):
    nc = bacc.Bacc("TRN2", debug=False, target_bir_lowering=False, num_devices=8)

    qT_d = nc.dram_tensor("qT", [NJ, P, 8, TB], FP8, kind="ExternalInput")
    kT_d = nc.dram_tensor("kT", [NC, P, 8, P], FP8, kind="ExternalInput")
    v_d = nc.dram_tensor("v", [P, NPAIR, 2, 1024], FP8, kind="ExternalInput")
    mk_d = nc.dram_tensor("masks", [2, P, 2, TB], FP16, kind="ExternalInput")
    out_d = nc.dram_tensor("out_loc", [2048, D], FP16, kind="ExternalOutput")

    with tile.TileContext(nc) as tc:
        with (
            tc.tile_pool(name="vres", bufs=1) as vpool,
            tc.tile_pool(name="kres", bufs=1) as krespool,
            tc.tile_pool(name="qt", bufs=2) as qpool,
            tc.tile_pool(name="attn", bufs=12) as apool,
            tc.tile_pool(name="mask", bufs=1) as mpool,
            tc.tile_pool(name="ostage", bufs=4) as opool,
            tc.tile_pool(name="ps_s", bufs=4, space=bass.MemorySpace.PSUM) as pspool,
            tc.tile_pool(name="ps_o", bufs=1, space=bass.MemorySpace.PSUM) as popool,
        ):
            v_sb = {}

            def get_v(pp):
                # lazy one-time load so early t-blocks' inputs win the DMA queue
                if pp not in v_sb:
                    vt = vpool.tile([P, 2, 1024], FP8, tag=f"v{pp}", name=f"v{pp}")
                    if pp == 0:
                        # split so mm2(J0,pp0,dd0) can start off the first half
                        for dd in range(2):
                            nc.sync.dma_start(
                                out=vt[:, :, dd * 512:(dd + 1) * 512],
                                in_=v_d[:, pp, :, dd * 512:(dd + 1) * 512],
                            )
                    else:
                        nc.sync.dma_start(out=vt[:], in_=v_d[:, pp])
                    v_sb[pp] = vt
                return v_sb[pp]

            k_sb = {}

            def get_k(c):
                if c not in k_sb:
                    kt = krespool.tile([P, 8, P], FP8, tag=f"k{c}", name=f"k{c}")
                    if c == 0:
                        # split the first k chunk in two so mm1(J0,c0,ccp01)
                        # can start as soon as the first 64KB lands
                        for p in range(2):
                            nc.gpsimd.dma_start(
                                out=kt[:, 4 * p:4 * p + 4, :],
                                in_=kT_d[c][:, 4 * p:4 * p + 4, :],
                            )
                    else:
                        nc.gpsimd.dma_start(out=kt[:], in_=kT_d[c])
                    k_sb[c] = kt
                return k_sb[c]

            masks = []
            for mi in range(2):
                mt = mpool.tile([P, 2, TB], FP16, tag=f"mask{mi}")
                nc.scalar.dma_start(out=mt[:], in_=mk_d[mi])
                masks.append(mt)

            import contextlib
            if reps > 1:
                for c in range(NC):
                    get_k(c)
                for pp in range(NPAIR):
                    get_v(pp)
            rep_ctx = tc.For_i(0, reps, 1) if reps > 1 else contextlib.nullcontext()
            with rep_ctx:
                _kernel_body(nc, tc, qT_d, get_k, get_v, out_d, masks,
                             qpool, apool, opool, pspool, popool)

    nc.compile()
    return nc


def _kernel_body(nc, tc, qT_d, get_k, get_v, out_d, masks,
                 qpool, apool, opool, pspool, popool):
    def emit_out(J, tt, accs, opool, out_d):
        ot = opool.tile([P, 1024], FP16, tag="ostage")
        for dd in range(2):
            if J == NJ - 1 and tt == 1 and dd == 1:
                # tail: drain the very last accumulator on the (now idle)
                # scalar engine so both copies run in parallel
                nc.scalar.activation(
                    ot[:, dd * 512:(dd + 1) * 512], accs[tt * 2 + dd][:],
                    mybir.ActivationFunctionType.Copy,
                )
            else:
                nc.vector.tensor_copy(
                    ot[:, dd * 512:(dd + 1) * 512], accs[tt * 2 + dd][:]
                )
        # the last block's stores ride the sync queue: its input loads are all
        # issued by then, and sync's end-of-kernel drain is ~3us cheaper than
        # gpsimd's
        q = nc.sync if J == NJ - 1 else nc.gpsimd
        q.dma_start(
            out=out_d[J * TB + tt * 128: J * TB + (tt + 1) * 128, :],
            in_=ot[:],
        )

    for J in range(NJ):
        qt = qpool.tile([P, 8, TB], FP8, tag="qt")
        if J == 0:
            for p in range(2):
                nc.sync.dma_start(
                    out=qt[:, 4 * p:4 * p + 4, :],
                    in_=qT_d[J][:, 4 * p:4 * p + 4, :],
                )
        else:
            nc.sync.dma_start(out=qt[:], in_=qT_d[J])
        ns = 4 * J + 4
        # pair order: the two DIAGONAL pairs first.  Their attn tiles need
        # ACT + a DVE mask-mul (~1.1us) before mm2 can read them — longer
        # than a pair's mm1 (~0.9us) — so running them while 2J more pairs
        # of mm1 remain hides that latency instead of stalling the PE.
        order = [2 * J, 2 * J + 1] + list(range(2 * J))
        # prefetch: k chunks for the NEXT t-block on the gpsimd queue (ahead
        # of this block's out-DMA triggers), this block's v pairs on sync.
        for pp in order:
            get_k(2 * pp)
            get_k(2 * pp + 1)
        for pp in order:
            get_v(pp)
        if J + 1 < NJ:
            for c in range(4 * J + 4, 4 * J + 8):
                get_k(c)
        # chunk pair 2J+1 (chunks 4J+2, 4J+3) is fully causal-masked for the
        # tt=0 row half (local rows 0..127 <-> global 2m+h < 256): skip its
        # tt=0 matmuls.
        accs = []
        for i in range(4):
            acc_t = popool.tile([P, 512], F32, tag=f"acc{i}", name=f"acc{i}_{J}")
            accs.append(acc_t)
        last = order[-1]
        # chunk PAIRS share one [128, 2, TB] scores psum tile: a PSUM zero
        # region is the whole 2KB bank, so one accumulation group (start on
        # the first matmul of the pair) covers both halves.  The [s, half, t]
        # layout doubles as the DoubleRow interleave for mm2.
        for pp in order:
            ps = pspool.tile([P, 2, TB], F32, tag="scores")
            for half in range(2):
                c = 2 * pp + half
                kt = get_k(c)
                for ccp in range(4):
                    nc.tensor.matmul(
                        ps[:, half, :],
                        kt[:, 2 * ccp:2 * ccp + 2, :],
                        qt[:, 2 * ccp:2 * ccp + 2, :],
                        start=(half == 0 and ccp == 0),
                        stop=(half == 1 and ccp == 3),
                        perf_mode=mybir.MatmulPerfMode.DoubleRow,
                    )
            at = apool.tile([P, 2, TB], FP8, tag="attn")
            nc.scalar.activation(
                at[:], ps[:],
                mybir.ActivationFunctionType.Tanh,
                scale=ACT_SCALE / 2,
            )
            if pp == 2 * J:
                nc.vector.tensor_mul(at[:], at[:], masks[0][:])
            elif pp == 2 * J + 1:
                nc.vector.tensor_mul(at[:], at[:], masks[1][:])
            vt = get_v(pp)
            tts = (0, 1) if pp <= 2 * J else (1,)
            for tt in tts:
                for dd in range(2):
                    nc.tensor.matmul(
                        accs[tt * 2 + dd][:],
                        at[:, :, tt * 128:(tt + 1) * 128],
                        vt[:, :, dd * 512:(dd + 1) * 512],
                        start=(pp == order[0]),
                        stop=(pp == (last if J > 0 else (2 * J if tt == 0 else 2 * J + 1))),
                        perf_mode=mybir.MatmulPerfMode.DoubleRow,
                    )
            if pp == (last if J > 0 else 2 * J):
                emit_out(J, 0, accs, opool, out_d)
        emit_out(J, 1, accs, opool, out_d)


def _get_nc(reps=1):
    key = ("nc", reps)
    if key not in _nc_cache:
        _nc_cache[key] = _build_nc(reps)
    return _nc_cache[key]


def _sign_vec(w):
    w = np.asarray(w, np.float32)
    alpha = np.float32(np.mean(np.abs(w), dtype=np.float32))
    hard = (alpha * np.sign(w)).astype(np.float32)
    hard = np.where(hard == 0, alpha, hard).astype(np.float32)
    return hard


def _rows_of(h):
    l = np.arange(2048)
    return 512 * (l // 256) + 2 * (l % 256) + h


def _masks_of(h):
    m = np.arange(TB)[None, :]      # local row in t-block
    p = np.arange(P)[:, None]       # s within chunk
    out = np.empty((4, P, TB), np.float32)
    for mi in range(4):
        out[mi] = ((2 * m + h) >= (128 * mi + p)).astype(np.float32)
    # paired layout matching the chunk-pair attn tiles: [2, P, 2, TB] with
    # pair 0 = diag chunks (mi0|mi1), pair 1 = (mi2|mi3)
    return np.stack(
        [np.stack([out[0], out[1]], axis=1),
         np.stack([out[2], out[3]], axis=1)], axis=0)


def kernel(x, bv_q, bv_k, bv_v):
    x = np.ascontiguousarray(np.asarray(x, np.float32))
    sq = _sign_vec(bv_q)
    sk = _sign_vec(bv_k)
    sv = _sign_vec(bv_v)

    k_full = (x * sk).astype(np.float32)
    v_full = (x * sv).astype(np.float32)

    q_s = (x * (sq * QK_SCALE)).astype(np.float32)
    k_s = (x * (sk * QK_SCALE)).astype(np.float32)

    # exact host-side half of the sigmoid split: 0.5 * cumsum(v)
    prefix_v = np.cumsum(v_full, axis=1, dtype=np.float64).astype(np.float32)

    nc = _get_nc()
    rows = {h: _rows_of(h) for h in range(2)}
    mks = {h: _masks_of(h) for h in range(2)}

    in_maps = []
    for core in range(8):
        b, h = core // 2, core % 2
        qrows = q_s[b][rows[h]]                         # [2048, 1024]
        qT_host = np.ascontiguousarray(
            qrows.reshape(NJ, TB, 8, P).transpose(0, 3, 2, 1)
        )                                               # [NJ, P, 8, TB]
        kT_host = np.ascontiguousarray(
            k_s[b].reshape(NC, P, 8, P).transpose(0, 3, 2, 1)
        )                                               # [NC, P, 8, P]
        v_host = np.ascontiguousarray(
            v_full[b].reshape(NPAIR, 2, P, 1024).transpose(2, 0, 1, 3)
        )                                               # [P, NPAIR, 2, 1024]
        in_maps.append({
            "qT": qT_host.astype(NP_FP8),
            "kT": kT_host.astype(NP_FP8),
            "v": v_host.astype(NP_FP8),
            "masks": mks[h].astype(NP_FP16),
        })

    bkr = run_bass_kernel_spmd(nc, in_maps, list(range(8)), trace=TRACE)
    _nc_cache["last"] = bkr
    res = bkr.results

    out = np.empty((B, T, D), np.float32)
    for core in range(8):
        b, h = core // 2, core % 2
        H = np.asarray(res[core]["out_loc"]).astype(np.float32)
        out[b, rows[h]] = 0.5 * (prefix_v[b][rows[h]] + H)

    return out, k_full, v_full


# revision 18
# speedup vs baseline: 1.0109x; 1.0086x over previous
"""HDC sigmoid-attention kernel for Trainium2 (8 NeuronCores).

Problem: out = causal_sigmoid_attn(q, k, v) where q/k/v = x * sign_vec(bv_*),
x: [4, 4096, 1024] f32.  Returns (out, k, v) like the reference.

Sharding: 8 cores = 4 batches x 2 row-parity groups.  Core (b, h) handles
batch b, rows {t : t % 2 == h}.  Row-parity interleaving makes the causal
work profile identical on every core, so one SPMD program serves all 8.

Per core: 2048 rows as 8 t-blocks (J=0..7) of 256 local rows; t-block J
covers global rows {512J + 2m + h}.  Causal extent of block J is s-chunks
0..4J+3 (chunk = 128 s values); the top 4 chunks are diagonal and get a
0/1 mask (host-precomputed, J-independent thanks to the parity trick).

Matmul 1 (scores^T) in FP8 e4m3 with DoubleRow (2 d-subtiles per pass,
~2x the bf16 row rate):
  psum[s=128, t=256] += kT[d=128, 2, s=128].T @ qT[d=128, 2, t=256]
q/k are scaled by 16*alpha on host before the e4m3 cast; the 1/(16*16)
plus the reference's 4/sqrt(D) fold into the activation scale.

Sigmoid split: sigma(z) = 0.5 + 0.5*tanh(z/2).  The 0.5*cumsum(v) term is
exact host-side work (prefix sum over s); the device only computes
H = sum_s tanh(z/2) * v.  tanh in [-1,1] is ~4x smaller in RMS than
sigma in [0,1], so fp8-quantizing BOTH mm2 operands costs only ~1% rel
err (vs ~2.5% without the split).  ACT emits tanh directly as e4m3.

Matmul 2 (H) in FP8 e4m3 DoubleRow over chunk PAIRS (2 s-chunks = 256
contraction per pass):
  psum[t=128, d=512] += gT[s=128, 2, t=128].T @ v8[s=128, 2, d=512]
The attn tile's natural [s, pair-half, t] layout IS the DoubleRow
interleave; v8 is host-packed as [s, pair, 2, d].  Host output:
out = 0.5 * (cumsum(v) + H).

kT (fp8, 4MB) and v8 (fp8, 4MB) are fully SBUF-resident; no streaming.
H is stored fp16 and combined on host in f32 (rel-err impact ~1e-4).
"""

import numpy as np
import ml_dtypes

import concourse.bass as bass
import concourse.bacc as bacc
import concourse.mybir as mybir
import concourse.tile as tile
from concourse.bass_utils import run_bass_kernel_spmd

B, T, D = 4, 4096, 1024
P = 128
NJ = 8          # t-blocks per core
TB = 256        # local rows per t-block
NC = 32         # s-chunks per batch
NPAIR = NC // 2

F32 = mybir.dt.float32
BF16 = mybir.dt.bfloat16
FP16 = mybir.dt.float16
FP8 = mybir.dt.float8e4
NP_FP16 = np.float16
NP_FP8 = ml_dtypes.float8_e4m3

QK_SCALE = 16.0                       # host pre-scale before e4m3 cast
ACT_SCALE = 0.125 / (QK_SCALE * QK_SCALE)

_nc_cache = {}
TRACE = False  # set True (e.g. from test.py) to collect an NTFF profile


def _build_nc(reps=1):
    nc = bacc.Bacc("TRN2", debug=False, target_bir_lowering=False, num_devices=8)

    qT_d = nc.dram_tensor("qT", [NJ, P, 8, TB], FP8, kind="ExternalInput")
    # kT is host-pre-interleaved for DoubleRowSwInterleave: per partition d,
    # columns [A127, B127, A126, B126, ..., A0, B0] where A/B are the two
    # d-subtiles of a DoubleRow pair (ccp), s-index reversed.
    kT_d = nc.dram_tensor("kT", [NC, P, 4, 2 * P], FP8, kind="ExternalInput")
    v_d = nc.dram_tensor("v", [P, NPAIR, 2, 1024], FP8, kind="ExternalInput")
    mk_d = nc.dram_tensor("masks", [2, P, 2, TB], FP16, kind="ExternalInput")
    out_d = nc.dram_tensor("out_loc", [2048, D], FP16, kind="ExternalOutput")

    with tile.TileContext(nc) as tc:
        with (
            tc.tile_pool(name="vres", bufs=1) as vpool,
            tc.tile_pool(name="kres", bufs=1) as krespool,
            tc.tile_pool(name="qt", bufs=2) as qpool,
            tc.tile_pool(name="attn", bufs=12) as apool,
            tc.tile_pool(name="mask", bufs=1) as mpool,
            tc.tile_pool(name="ostage", bufs=4) as opool,
            tc.tile_pool(name="ps_s", bufs=4, space=bass.MemorySpace.PSUM) as pspool,
            tc.tile_pool(name="ps_o", bufs=1, space=bass.MemorySpace.PSUM) as popool,
        ):
            v_sb = {}

            def get_v(pp):
                # lazy one-time load so early t-blocks' inputs win the DMA queue
                if pp not in v_sb:
                    vt = vpool.tile([P, 2, 1024], FP8, tag=f"v{pp}", name=f"v{pp}")
                    if pp == 0:
                        # split so mm2(J0,pp0,dd0) can start off the first half
                        for dd in range(2):
                            nc.sync.dma_start(
                                out=vt[:, :, dd * 512:(dd + 1) * 512],
                                in_=v_d[:, pp, :, dd * 512:(dd + 1) * 512],
                            )
                    else:
                        nc.sync.dma_start(out=vt[:], in_=v_d[:, pp])
                    v_sb[pp] = vt
                return v_sb[pp]

            k_sb = {}

            def get_k(c):
                if c not in k_sb:
                    kt = krespool.tile([P, 4, 2 * P], FP8, tag=f"k{c}", name=f"k{c}")
                    if c == 0:
                        # split the first k chunk in two so mm1(J0,c0,ccp01)
                        # can start as soon as the first 64KB lands
                        for p in range(2):
                            nc.gpsimd.dma_start(
                                out=kt[:, 2 * p:2 * p + 2, :],
                                in_=kT_d[c][:, 2 * p:2 * p + 2, :],
                            )
                    else:
                        nc.gpsimd.dma_start(out=kt[:], in_=kT_d[c])
                    k_sb[c] = kt
                return k_sb[c]

            masks = []
            for mi in range(2):
                mt = mpool.tile([P, 2, TB], FP16, tag=f"mask{mi}")
                nc.scalar.dma_start(out=mt[:], in_=mk_d[mi])
                masks.append(mt)

            import contextlib
            if reps > 1:
                for c in range(NC):
                    get_k(c)
                for pp in range(NPAIR):
                    get_v(pp)
            rep_ctx = tc.For_i(0, reps, 1) if reps > 1 else contextlib.nullcontext()
            with rep_ctx:
                _kernel_body(nc, tc, qT_d, get_k, get_v, out_d, masks,
                             qpool, apool, opool, pspool, popool)

    nc.compile()
    return nc


def _kernel_body(nc, tc, qT_d, get_k, get_v, out_d, masks,
                 qpool, apool, opool, pspool, popool):
    def emit_out(J, tt, accs, opool, out_d):
        ot = opool.tile([P, 1024], FP16, tag="ostage")
        for dd in range(2):
            if J == NJ - 1 and tt == 1 and dd == 1:
                # tail: drain the very last accumulator on the (now idle)
                # scalar engine so both copies run in parallel
                nc.scalar.activation(
                    ot[:, dd * 512:(dd + 1) * 512], accs[tt * 2 + dd][:],
                    mybir.ActivationFunctionType.Copy,
                )
            else:
                nc.vector.tensor_copy(
                    ot[:, dd * 512:(dd + 1) * 512], accs[tt * 2 + dd][:]
                )
        # the last block's stores ride the sync queue: its input loads are all
        # issued by then, and sync's end-of-kernel drain is ~3us cheaper than
        # gpsimd's
        q = nc.sync if J == NJ - 1 else nc.gpsimd
        q.dma_start(
            out=out_d[J * TB + tt * 128: J * TB + (tt + 1) * 128, :],
            in_=ot[:],
        )

    for J in range(NJ):
        qt = qpool.tile([P, 8, TB], FP8, tag="qt")
        if J == 0:
            for p in range(2):
                nc.sync.dma_start(
                    out=qt[:, 4 * p:4 * p + 4, :],
                    in_=qT_d[J][:, 4 * p:4 * p + 4, :],
                )
        else:
            nc.sync.dma_start(out=qt[:], in_=qT_d[J])
        ns = 4 * J + 4
        # pair order: the two DIAGONAL pairs first.  Their attn tiles need
        # ACT + a DVE mask-mul (~1.1us) before mm2 can read them — longer
        # than a pair's mm1 (~0.9us) — so running them while 2J more pairs
        # of mm1 remain hides that latency instead of stalling the PE.
        order = [2 * J, 2 * J + 1] + list(range(2 * J))
        # prefetch: k chunks for the NEXT t-block on the gpsimd queue (ahead
        # of this block's out-DMA triggers), this block's v pairs on sync.
        for pp in order:
            get_k(2 * pp)
            get_k(2 * pp + 1)
        for pp in order:
            get_v(pp)
        if J + 1 < NJ:
            for c in range(4 * J + 4, 4 * J + 8):
                get_k(c)
        # chunk pair 2J+1 (chunks 4J+2, 4J+3) is fully causal-masked for the
        # tt=0 row half (local rows 0..127 <-> global 2m+h < 256): skip its
        # tt=0 matmuls.
        accs = []
        for i in range(4):
            acc_t = popool.tile([P, 512], F32, tag=f"acc{i}", name=f"acc{i}_{J}")
            accs.append(acc_t)
        last = order[-1]
        # chunk PAIRS share one [128, 2, TB] scores psum tile: a PSUM zero
        # region is the whole 2KB bank, so one accumulation group (start on
        # the first matmul of the pair) covers both halves.  The [s, half, t]
        # layout doubles as the DoubleRow interleave for mm2.
        for pp in order:
            ps = pspool.tile([P, 2, TB], F32, tag="scores")
            for half in range(2):
                c = 2 * pp + half
                kt = get_k(c)
                for ccp in range(4):
                    nc.tensor.matmul(
                        ps[:, half, :],
                        kt[:, ccp, :],
                        qt[:, 2 * ccp:2 * ccp + 2, :],
                        start=(half == 0 and ccp == 0),
                        stop=(half == 1 and ccp == 3),
                        perf_mode=mybir.MatmulPerfMode.DoubleRowSwInterleave,
                    )
            at = apool.tile([P, 2, TB], FP8, tag="attn")
            nc.scalar.activation(
                at[:], ps[:],
                mybir.ActivationFunctionType.Tanh,
                scale=ACT_SCALE / 2,
            )
            if pp == 2 * J:
                nc.vector.tensor_mul(at[:], at[:], masks[0][:])
            elif pp == 2 * J + 1:
                nc.vector.tensor_mul(at[:], at[:], masks[1][:])
            vt = get_v(pp)
            tts = (0, 1) if pp <= 2 * J else (1,)
            for tt in tts:
                for dd in range(2):
                    nc.tensor.matmul(
                        accs[tt * 2 + dd][:],
                        at[:, :, tt * 128:(tt + 1) * 128],
                        vt[:, :, dd * 512:(dd + 1) * 512],
                        start=(pp == order[0]),
                        stop=(pp == (last if J > 0 else (2 * J if tt == 0 else 2 * J + 1))),
                        perf_mode=mybir.MatmulPerfMode.DoubleRow,
                    )
            if pp == (last if J > 0 else 2 * J):
                emit_out(J, 0, accs, opool, out_d)
        emit_out(J, 1, accs, opool, out_d)


def _get_nc(reps=1):
    key = ("nc", reps)
    if key not in _nc_cache:
        _nc_cache[key] = _build_nc(reps)
    return _nc_cache[key]


def _sign_vec(w):
    w = np.asarray(w, np.float32)
    alpha = np.float32(np.mean(np.abs(w), dtype=np.float32))
    hard = (alpha * np.sign(w)).astype(np.float32)
    hard = np.where(hard == 0, alpha, hard).astype(np.float32)
    return hard


def _rows_of(h):
    l = np.arange(2048)
    return 512 * (l // 256) + 2 * (l % 256) + h


def _masks_of(h):
    m = np.arange(TB)[None, :]      # local row in t-block
    p = np.arange(P)[:, None]       # s within chunk
    out = np.empty((4, P, TB), np.float32)
    for mi in range(4):
        out[mi] = ((2 * m + h) >= (128 * mi + p)).astype(np.float32)
    # paired layout matching the chunk-pair attn tiles: [2, P, 2, TB] with
    # pair 0 = diag chunks (mi0|mi1), pair 1 = (mi2|mi3)
    return np.stack(
        [np.stack([out[0], out[1]], axis=1),
         np.stack([out[2], out[3]], axis=1)], axis=0)


def kernel(x, bv_q, bv_k, bv_v):
    x = np.ascontiguousarray(np.asarray(x, np.float32))
    sq = _sign_vec(bv_q)
    sk = _sign_vec(bv_k)
    sv = _sign_vec(bv_v)

    k_full = (x * sk).astype(np.float32)
    v_full = (x * sv).astype(np.float32)

    q_s = (x * (sq * QK_SCALE)).astype(np.float32)
    k_s = (x * (sk * QK_SCALE)).astype(np.float32)

    # exact host-side half of the sigmoid split: 0.5 * cumsum(v)
    prefix_v = np.cumsum(v_full, axis=1, dtype=np.float64).astype(np.float32)

    nc = _get_nc()
    rows = {h: _rows_of(h) for h in range(2)}
    mks = {h: _masks_of(h) for h in range(2)}

    in_maps = []
    for core in range(8):
        b, h = core // 2, core % 2
        qrows = q_s[b][rows[h]]                         # [2048, 1024]
        qT_host = np.ascontiguousarray(
            qrows.reshape(NJ, TB, 8, P).transpose(0, 3, 2, 1)
        )                                               # [NJ, P, 8, TB]
        kT8 = k_s[b].reshape(NC, P, 8, P).transpose(0, 3, 2, 1)  # [NC, d, 8, s]
        # SwInterleave layout per (chunk, ccp): [d, s-reversed, {A,B}]
        kT_host = np.ascontiguousarray(
            kT8.reshape(NC, P, 4, 2, P)[:, :, :, :, ::-1].transpose(0, 1, 2, 4, 3)
            .reshape(NC, P, 4, 2 * P)
        )                                               # [NC, P, 4, 256]
        v_host = np.ascontiguousarray(
            v_full[b].reshape(NPAIR, 2, P, 1024).transpose(2, 0, 1, 3)
        )                                               # [P, NPAIR, 2, 1024]
        in_maps.append({
            "qT": qT_host.astype(NP_FP8),
            "kT": kT_host.astype(NP_FP8),
            "v": v_host.astype(NP_FP8),
            "masks": mks[h].astype(NP_FP16),
        })

    bkr = run_bass_kernel_spmd(nc, in_maps, list(range(8)), trace=TRACE)
    _nc_cache["last"] = bkr
    res = bkr.results

    out = np.empty((B, T, D), np.float32)
    for core in range(8):
        b, h = core // 2, core % 2
        H = np.asarray(res[core]["out_loc"]).astype(np.float32)
        out[b, rows[h]] = 0.5 * (prefix_v[b][rows[h]] + H)

    return out, k_full, v_full
